# revision 9
# baseline (speedup 1.0000x reference)
"""Multi-head attention (B=2, N=2048, C=1024, H=16, D=64) on 8 trn2 cores.

Sharding: core c -> (batch b = c//4, head-group g = c%4 covering 4 heads).
Tensor-parallel over heads: Wq/Wk/Wv split column-wise, Wo row-wise; the
4 partial outputs per batch are summed on the host (+ bias).

Device layout trick: the host feeds activations TRANSPOSED ([C, seq]) so
every matmul on chip has its contraction dim on partitions with no
on-chip transposes:
  QT/KT panels [f, seq]  (projection outputs, transposed orientation)
  VP panel     [seq, f]  (natural orientation, +ones column per head)
  S^T  = Kh @ QhT        [sk, sq]  (d=64 contraction, 2-head row-packed)
  P^T  = exp(S^T * scale)          (ScalarE, reads PSUM directly)
  O'^T = [Vh|1]T-style   [65, sq]  (row 64 = softmax denominator)
  Y^T  = Wo^T @ (O^T/den)[o, seq]
All matmuls run as float32r (full-rate fp32-storage mode).
"""

import os
import sys

import numpy as np

sys.path.insert(0, "/opt/trn_rl_repo")

import concourse.bacc as bacc
import concourse.bass as bass
import concourse.tile as tile
from concourse import mybir
from concourse.bass_utils import run_bass_kernel_spmd

F32 = mybir.dt.float32
F32R = mybir.dt.float32r

B = 2
SEQ = 2048
C = 1024
NH = 4          # heads per core
D = 64
FH = NH * D     # 256: feature slice per core
SCALE = D ** -0.5

N_CORES = 8
CCN = C // 128      # 8 contraction chunks
SQN = SEQ // 512    # 4 query chunks
SKN = SEQ // 128    # 16 key chunks

LAST_RESULTS = None  # stash for test harness introspection


def build_kernel(tc, qT, kT, vT, wq, wk, wv, wo, yT):
    nc = tc.nc

    with (
        tc.tile_pool(name="weights", bufs=1) as wpool,
        tc.tile_pool(name="panels", bufs=1) as panels,
        tc.tile_pool(name="xin", bufs=8) as xpool,
        tc.tile_pool(name="ptile", bufs=3) as ppool,
        tc.tile_pool(name="otile", bufs=2) as opool,
        tc.tile_pool(name="ytile", bufs=3) as ypool,
        tc.tile_pool(name="small", bufs=4) as small,
    ):
        # ---- resident weights ----
        wq_sb = wpool.tile([128, CCN, FH], F32R, name="wq_sb", tag="wq")
        wk_sb = wpool.tile([128, CCN, FH], F32R, name="wk_sb", tag="wk")
        wv_sb = wpool.tile([128, CCN, FH], F32R, name="wv_sb", tag="wv")
        wo_sb = wpool.tile([128, 2, C], F32R, name="wo_sb", tag="wo")
        nc.sync.dma_start(out=wq_sb, in_=wq[:, :].rearrange("(n p) m -> p n m", p=128))
        nc.sync.dma_start(out=wk_sb, in_=wk[:, :].rearrange("(n p) m -> p n m", p=128))
        nc.sync.dma_start(out=wv_sb, in_=wv[:, :].rearrange("(n p) m -> p n m", p=128))
        nc.sync.dma_start(out=wo_sb, in_=wo[:, :].rearrange("(n p) m -> p n m", p=128))

        # ---- persistent activation panels ----
        qt_sb = panels.tile([128, 2, SEQ], F32R, name="qt_sb", tag="qt")   # [p, fc, sq] = QT
        kt_sb = panels.tile([128, 2, SEQ], F32R, name="kt_sb", tag="kt")   # [p, fc, sk] = KT
        vp_sb = panels.tile([128, SKN, NH, D + 1], F32R, name="vp_sb", tag="vp")  # V' natural
        nc.vector.memset(vp_sb[:, :, :, D:D + 1].bitcast(F32), 1.0)

        # ---- projections (PSUM: 8 banks of [128,512] accumulators) ----
        with tc.tile_pool(name="ps_proj", bufs=8, space="PSUM") as ps_proj:
            # Q then K: out panel [f, seq]; stationary = W chunk, moving = xT.
            for name, src, w_sb, dst in (
                ("q", qT, wq_sb, qt_sb),
                ("k", kT, wk_sb, kt_sb),
            ):
                acc = {}
                for cc in range(CCN):
                    xin = xpool.tile([128, SEQ], F32R, name="xin", tag="xin")
                    nc.sync.dma_start(out=xin, in_=src[cc * 128:(cc + 1) * 128, :])
                    for fc in range(2):
                        for sqc in range(SQN):
                            if cc == 0:
                                acc[(fc, sqc)] = ps_proj.tile(
                                    [128, 512], F32, name="pacc", tag="pacc"
                                )
                            nc.tensor.matmul(
                                out=acc[(fc, sqc)],
                                lhsT=w_sb[:, cc, fc * 128:(fc + 1) * 128],
                                rhs=xin[:, sqc * 512:(sqc + 1) * 512],
                                start=(cc == 0),
                                stop=(cc == CCN - 1),
                            )
                for fc in range(2):
                    for sqc in range(SQN):
                        nc.vector.tensor_copy(
                            out=dst[:, fc, sqc * 512:(sqc + 1) * 512],
                            in_=acc[(fc, sqc)],
                        )

            # V: natural orientation; stationary = vT chunk, moving = Wv chunk.
            # All 8 vT chunks stay resident so each skc accumulates in its
            # own PSUM bank (start=True zeroes a whole bank's has_written).
            vx = []
            for cc in range(CCN):
                xin = xpool.tile([128, SEQ], F32R, name="xin", tag="xin")
                nc.sync.dma_start(out=xin, in_=vT[cc * 128:(cc + 1) * 128, :])
                vx.append(xin)
            for skc in range(SKN):
                vacc = ps_proj.tile([128, 256], F32, name="vacc", tag="pacc")
                for cc in range(CCN):
                    nc.tensor.matmul(
                        out=vacc,
                        lhsT=vx[cc][:, skc * 128:(skc + 1) * 128],
                        rhs=wv_sb[:, cc, :],
                        start=(cc == 0),
                        stop=(cc == CCN - 1),
                    )
                nc.vector.tensor_copy(
                    out=vp_sb[:, skc, :, 0:D],
                    in_=vacc.rearrange("p (h d) -> p h d", h=NH),
                )

        # ---- attention + output projection ----
        with (
            tc.tile_pool(name="ps_s", bufs=2, space="PSUM") as ps_s,
            tc.tile_pool(name="ps_o", bufs=2, space="PSUM") as ps_o,
            tc.tile_pool(name="ps_y", bufs=2, space="PSUM") as ps_y,
        ):
            for sqc in range(SQN):
                sq = slice(sqc * 512, (sqc + 1) * 512)
                ot_sb = opool.tile([128, 2, 512], F32R, name="ot", tag="ot")  # [f, fc, sq]
                for hp in range(2):  # head pair = fc chunk
                    o_ps = [
                        ps_o.tile([D + 1, 512], F32, name="oacc", tag="oacc") for _ in range(2)
                    ]
                    for skc in range(SKN):
                        sk = slice(skc * 128, (skc + 1) * 128)
                        s_ps = ps_s.tile([128, 1024], F32, name="sacc", tag="sacc")
                        # two heads row-packed into the 128-deep array
                        for h2 in range(2):
                            rows = slice(h2 * 64, (h2 + 1) * 64)
                            nc.tensor.matmul(
                                out=s_ps[:, h2 * 512:(h2 + 1) * 512],
                                lhsT=kt_sb[rows, hp, sk],
                                rhs=qt_sb[rows, hp, sq],
                                start=True,
                                stop=True,
                            )
                        p_sb = ppool.tile([128, 1024], F32R, name="p", tag="p")
                        nc.scalar.activation(
                            out=p_sb,
                            in_=s_ps[:, :],
                            func=mybir.ActivationFunctionType.Exp,
                            scale=SCALE,
                        )
                        for h2 in range(2):
                            nc.tensor.matmul(
                                out=o_ps[h2],
                                lhsT=vp_sb[:, skc, hp * 2 + h2, :],
                                rhs=p_sb[:, h2 * 512:(h2 + 1) * 512],
                                start=(skc == 0),
                                stop=(skc == SKN - 1),
                            )
                    # normalize: rows 0..63 = O^T, row 64 = sum(exp)
                    for h2 in range(2):
                        rec = small.tile([1, 512], F32, name="rec", tag="rec")
                        nc.vector.reciprocal(rec, o_ps[h2][D:D + 1, :])
                        # replicate across partitions (DVE can't broadcast)
                        rec_b = small.tile([D, 512], F32, name="recb", tag="recb")
                        nc.gpsimd.partition_broadcast(rec_b, rec)
                        if h2 == 0:
                            nc.vector.tensor_mul(
                                out=ot_sb[0:D, hp, :],
                                in0=o_ps[h2][0:D, :],
                                in1=rec_b,
                            )
                        else:
                            tmp = small.tile([D, 512], F32R, name="otmp", tag="otmp")
                            nc.vector.tensor_mul(
                                out=tmp, in0=o_ps[h2][0:D, :], in1=rec_b
                            )
                            # cross-partition move (DVE lanes can't shift)
                            nc.sync.dma_start(out=ot_sb[D:128, hp, :], in_=tmp)

                # output projection for this query chunk
                for oc in range(8):
                    y_ps = ps_y.tile([128, 512], F32, name="yacc", tag="yacc")
                    for fc in range(2):
                        nc.tensor.matmul(
                            out=y_ps,
                            lhsT=wo_sb[:, fc, oc * 128:(oc + 1) * 128],
                            rhs=ot_sb[:, fc, :],
                            start=(fc == 0),
                            stop=(fc == 1),
                        )
                    y_sb = ypool.tile([128, 512], F32, name="y", tag="y")
                    nc.vector.tensor_copy(out=y_sb, in_=y_ps)
                    nc.sync.dma_start(
                        out=yT[oc * 128:(oc + 1) * 128, sq], in_=y_sb
                    )


def build_bass():
    nc = bacc.Bacc("TRN2", target_bir_lowering=False, debug=False,
                   enable_asserts=False)
    qT = nc.dram_tensor("qT", [C, SEQ], F32R, kind="ExternalInput").ap()
    kT = nc.dram_tensor("kT", [C, SEQ], F32R, kind="ExternalInput").ap()
    vT = nc.dram_tensor("vT", [C, SEQ], F32R, kind="ExternalInput").ap()
    wq = nc.dram_tensor("wq", [C, FH], F32R, kind="ExternalInput").ap()
    wk = nc.dram_tensor("wk", [C, FH], F32R, kind="ExternalInput").ap()
    wv = nc.dram_tensor("wv", [C, FH], F32R, kind="ExternalInput").ap()
    wo = nc.dram_tensor("wo", [FH, C], F32R, kind="ExternalInput").ap()
    yT = nc.dram_tensor("yT", [C, SEQ], F32, kind="ExternalOutput").ap()
    with tile.TileContext(nc) as tc:
        build_kernel(tc, qT, kT, vT, wq, wk, wv, wo, yT)
    nc.compile()
    return nc


_NC = None


def _get_nc():
    global _NC
    if _NC is None:
        _NC = build_bass()
    return _NC


def make_in_maps(q, k, v, Wq, Wk, Wv, Wo):
    in_maps = []
    for c in range(N_CORES):
        b, g = divmod(c, 4)
        fs = slice(g * FH, (g + 1) * FH)
        in_maps.append(dict(
            qT=np.ascontiguousarray(q[b].T),
            kT=np.ascontiguousarray(k[b].T),
            vT=np.ascontiguousarray(v[b].T),
            wq=np.ascontiguousarray(Wq[:, fs]),
            wk=np.ascontiguousarray(Wk[:, fs]),
            wv=np.ascontiguousarray(Wv[:, fs]),
            wo=np.ascontiguousarray(Wo[fs, :]),
        ))
    return in_maps


def kernel(q, k, v, Wq, Wk, Wv, Wo, bo):
    global LAST_RESULTS
    q = np.asarray(q, dtype=np.float32)
    k = np.asarray(k, dtype=np.float32)
    v = np.asarray(v, dtype=np.float32)
    Wq = np.asarray(Wq, dtype=np.float32)
    Wk = np.asarray(Wk, dtype=np.float32)
    Wv = np.asarray(Wv, dtype=np.float32)
    Wo = np.asarray(Wo, dtype=np.float32)
    bo = np.asarray(bo, dtype=np.float32)

    nc = _get_nc()
    in_maps = make_in_maps(q, k, v, Wq, Wk, Wv, Wo)
    res = run_bass_kernel_spmd(
        nc, in_maps, list(range(N_CORES)),
        trace=bool(os.environ.get("KERNEL_TRACE")),
    )
    LAST_RESULTS = res

    out = np.zeros((B, SEQ, C), dtype=np.float32)
    for c in range(N_CORES):
        out[c // 4] += res.results[c]["yT"].T
    out += bo
    return out.astype(np.float32)
